# revision 24
# baseline (speedup 1.0000x reference)
"""Causal multi-head attention Trainium2 kernel (8 NeuronCores).

Problem: B=4, L=2048, D=1024, 16 heads x (dh=64, dv=64), causal mask.
Sharding: data-parallel over batch (4) x tensor-parallel over heads (2 groups
of 8). Core c handles batch c//2, head-group c%2. Each core computes its
partial output projection (ctx_g @ Wo_g); the host sums the two head-group
partials per batch and adds the bias.

v3: bf16 everywhere on SBUF (PSUM accumulates f32), x pre-transposed on the
host into [128, 8, L] d-major layout so the kernel has no PE transposes.
Loop order is q-chunk-outer: for each 512-query chunk j the four head-pairs
run flash-style attention (S^T = K@Q^T per 128-k-tile, exp on ACT with the
1/8 scale folded in, tril mask on the diagonal tiles, PV accumulation with
a ones-column of V giving the softmax denominator in PSUM row 64), while
the Q/K/V projections of chunk j+1 and the output projections of earlier
chunks dribble one unit per attention group - placed between the S and PV
quads so every stationary load hides under a full matmul - keeping the PE
dense at 2.4 GHz. S^T and exp are trimmed to the causal region at
128-column granularity. Heater matmuls bridge the final softmax-normalize
chain so the output-projection epilogue (O(2) interleaved with O(3) to
cover the last normalize) runs at full clock. 300 us on HW, vs 422 us for
the f32r/transpose-on-device baseline.
"""

import numpy as np
from contextlib import ExitStack

import ml_dtypes

import concourse.bass as bass
import concourse.tile as tile
from concourse import bacc, mybir

F32 = mybir.dt.float32
BF16 = mybir.dt.bfloat16
AF = mybir.ActivationFunctionType

B, L, D = 4, 2048, 1024
N_HEAD, DH, DV = 16, 64, 64
N_CORES = 8
HPC = N_HEAD // 2          # heads per core (8)
OC = HPC * DH              # per-core projection width (512)
NHP = HPC // 2             # head-pairs per core (4)
NCH = L // 512             # q-chunks (4)
NLT = L // 128             # l-tiles (16)


def build_nc():
    nc = bacc.Bacc("TRN2", target_bir_lowering=False, debug=False,
                   num_devices=N_CORES)

    xt = nc.dram_tensor("xt", [128, 8, L], BF16, kind="ExternalInput").ap()
    wq = nc.dram_tensor("wq", [128, 8, OC], BF16, kind="ExternalInput").ap()
    wk = nc.dram_tensor("wk", [128, 8, OC], BF16, kind="ExternalInput").ap()
    wv = nc.dram_tensor("wv", [128, 8, OC], BF16, kind="ExternalInput").ap()
    wo = nc.dram_tensor("wo", [128, 4, D], BF16, kind="ExternalInput").ap()
    out = nc.dram_tensor("out", [L, D], F32, kind="ExternalOutput").ap()

    with tile.TileContext(nc) as tc, ExitStack() as ctx:
        top = ctx.enter_context(tc.tile_pool(name="top", bufs=1))
        psP = ctx.enter_context(tc.tile_pool(name="psP", bufs=2, space="PSUM"))
        psS = ctx.enter_context(tc.tile_pool(name="psS", bufs=2, space="PSUM"))
        psC = ctx.enter_context(tc.tile_pool(name="psC", bufs=2, space="PSUM"))
        phb = ctx.enter_context(tc.tile_pool(name="phb", bufs=2))
        pho = ctx.enter_context(tc.tile_pool(name="pho", bufs=3))

        xts = top.tile([128, 8, L], BF16)
        wqs = top.tile([128, 8, OC], BF16)
        wks = top.tile([128, 8, OC], BF16)
        wvs = top.tile([128, 8, OC], BF16)
        wos = top.tile([128, 4, D], BF16)
        qt = top.tile([128, NHP, L], BF16)
        kt = top.tile([128, NHP, L], BF16)
        # V: [128(k), ltile, head, 65] - col 64 is ones (softmax denominator)
        vt = top.tile([128, NLT, HPC, DV + 1], BF16)
        ct = top.tile([128, NHP, L], BF16)        # normalized ctx^T
        trilf = top.tile([128, 128], F32)
        tril = top.tile([128, 128], BF16)
        ones = top.tile([128, 1], BF16)
        ones_col = top.tile([1, DV], BF16)

        # input DMAs spread across four DGE queues so wq/wk/x0 land in
        # parallel (~4us) instead of serially on sync (~16us)
        nc.sync.dma_start(out=xts[:, :, 0:512], in_=xt[:, :, 0:512])
        nc.scalar.dma_start(out=wqs, in_=wq)
        nc.gpsimd.dma_start(out=wks, in_=wk)
        nc.scalar.dma_start(out=wvs, in_=wv)

        nc.vector.memset(ones, 1.0)
        nc.vector.memset(ones_col, 1.0)
        nc.vector.tensor_copy(
            vt[:, :, :, DV:DV + 1].rearrange("p t h c -> p (t h) c"),
            ones.broadcast_to((128, NLT * HPC, 1)))
        # causal keep-mask for S^T diag blocks: tril[k, q] = 1.0 iff q >= k
        nc.gpsimd.memset(trilf, 0.0)
        nc.gpsimd.affine_select(
            out=trilf, in_=trilf, compare_op=mybir.AluOpType.is_gt,
            fill=1.0, base=0, pattern=[[-1, 128]], channel_multiplier=1)
        nc.vector.tensor_copy(tril, trilf)

        # ---------------- projection / output units ----------------
        def qk_unit(c, hp, wsrc, dst):
            def run():
                pp = psP.tile([128, 512], F32, tag="pp", name="pp")
                for d in range(8):
                    nc.tensor.matmul(pp, wsrc[:, d, hp * 128:(hp + 1) * 128],
                                     xts[:, d, c * 512:(c + 1) * 512],
                                     start=(d == 0), stop=(d == 7))
                nc.vector.tensor_copy(dst[:, hp, c * 512:(c + 1) * 512], pp)
            return run

        def v_unit(lt):
            def run():
                pp = psP.tile([128, 512], F32, tag="pp", name="pp")
                for d in range(8):
                    nc.tensor.matmul(pp, xts[:, d, lt * 128:(lt + 1) * 128],
                                     wvs[:, d, :], start=(d == 0),
                                     stop=(d == 7))
                nc.vector.tensor_copy(
                    vt[:, lt, :, 0:DV],
                    pp.rearrange("p (h v) -> p h v", h=HPC))
            return run

        ost_map = {}

        def o_unit(lt, n):
            def run():
                if n == 0:
                    ost_map[lt] = pho.tile([128, D], F32, tag="ost",
                                           name="ost")
                ost = ost_map[lt]
                pp = psP.tile([128, 512], F32, tag="pp", name="pp")
                for v in range(4):
                    nc.tensor.matmul(pp, ct[:, v, lt * 128:(lt + 1) * 128],
                                     wos[:, v, n * 512:(n + 1) * 512],
                                     start=(v == 0), stop=(v == 3))
                nc.vector.tensor_copy(ost[:, n * 512:(n + 1) * 512], pp)
                if n == 1:
                    nc.sync.dma_start(out=out[lt * 128:(lt + 1) * 128, :],
                                      in_=ost)
                    del ost_map[lt]
            return run

        # ---------------- attention for one (head-pair, q-chunk) ----------
        def attention(hp, j, units):
            n_g = 2 * (j + 1)
            pctxs = {h: psC.tile([DV + 1, 512], F32, tag="pctx",
                                 name=f"pctx{h}") for h in range(2)}
            hist = {}
            for g in range(n_g + 1):
                cur = {}
                if g < n_g:
                    for h in range(2):
                        po = 64 * h
                        psc = psS.tile([128, 2, 512], F32, tag="psc",
                                       name=f"psc{h}")
                        pexp = phb.tile([128, 2, 512], BF16, tag="pexp",
                                        bufs=4, name=f"pexp{h}")
                        c0s = []
                        for r2 in range(2):
                            kt_i = 2 * g + r2
                            r = kt_i - 4 * j
                            c0 = 128 * r if r > 0 else 0
                            c0s.append(c0)
                            nc.tensor.matmul(
                                psc[:, r2, c0:512],
                                kt[po:po + DH, hp,
                                   kt_i * 128:(kt_i + 1) * 128],
                                qt[po:po + DH, hp,
                                   j * 512 + c0:(j + 1) * 512],
                                start=True, stop=True)
                        cm = min(c0s)
                        nc.scalar.activation(
                            pexp[:, :, cm:512], psc[:, :, cm:512],
                            AF.Exp, scale=0.125)
                        # mask the causal diagonal blocks right after exp
                        for r2 in range(2):
                            r = 2 * g + r2 - 4 * j
                            if r >= 0:
                                nc.vector.tensor_mul(
                                    pexp[:, r2, r * 128:(r + 1) * 128],
                                    pexp[:, r2, r * 128:(r + 1) * 128],
                                    tril)
                        cur[h] = pexp
                # one filler unit per group, between the S and PV quads, so
                # every stationary load hides under a full matmul
                if g < n_g:
                    hist[g] = cur
                if units:
                    units.pop(0)()
                pg = g - 1
                if pg in hist:
                    pex = hist.pop(pg)
                    for h in range(2):
                        H = 2 * hp + h
                        for r2 in range(2):
                            kt_i = 2 * pg + r2
                            r = kt_i - 4 * j
                            c0 = 128 * r if r > 0 else 0
                            nc.tensor.matmul(
                                pctxs[h][:, c0:512],
                                vt[:, kt_i, H, :],
                                pex[h][:, r2, c0:512],
                                start=(kt_i == 0), stop=(kt_i == 4 * j + 3))
            for h in range(2):
                po = 64 * h
                rs = phb.tile([1, 512], F32, tag="rs", name="rs")
                nc.vector.tensor_copy(rs, pctxs[h][DV:DV + 1, :])
                inv = phb.tile([1, 512], F32, tag="inv", name="inv")
                nc.vector.reciprocal_approx_fast(out=inv, in_=rs)
                bc = phb.tile([64, 512], F32, tag="bc", name="bc")
                nc.gpsimd.partition_broadcast(out_ap=bc, in_ap=inv)
                nc.vector.tensor_mul(
                    ct[po:po + DV, hp, j * 512:(j + 1) * 512],
                    pctxs[h][0:DV, :], bc)

        # ---------------- schedule ----------------
        # prologue: just enough of chunk 0 for attention(hp0) to start
        qk_unit(0, 0, wqs, qt)()
        qk_unit(0, 0, wks, kt)()
        for lt in range(4):
            v_unit(lt)()
        # remaining inputs dispatched only now, so the critical chunk-0
        # transfers above get the full DMA bandwidth
        for c in range(1, NCH):
            nc.sync.dma_start(out=xts[:, :, c * 512:(c + 1) * 512],
                              in_=xt[:, :, c * 512:(c + 1) * 512])
        nc.sync.dma_start(out=wos, in_=wo)

        for j in range(NCH):
            units = []
            if j == 0:
                for hp in range(1, NHP):
                    units.append(qk_unit(0, hp, wqs, qt))
                    units.append(qk_unit(0, hp, wks, kt))
                for hp in range(NHP):
                    units.append(qk_unit(1, hp, wqs, qt))
                for hp in range(NHP):
                    units.append(qk_unit(1, hp, wks, kt))
                for lt in range(4, 8):
                    units.append(v_unit(lt))
            elif j + 1 < NCH:
                for hp in range(NHP):
                    units.append(qk_unit(j + 1, hp, wqs, qt))
                    units.append(qk_unit(j + 1, hp, wks, kt))
                for lt in range(4 * (j + 1), 4 * (j + 2)):
                    units.append(v_unit(lt))
            if j >= 2:
                # O(j-2) late so drains land in the attention-heavy tail
                for lt in range(4 * (j - 2), 4 * (j - 1)):
                    units.append(o_unit(lt, 0))
                    units.append(o_unit(lt, 1))
            for hp in range(NHP):
                attention(hp, j, units)
            while units:
                units.pop(0)()

        # bridge the final normalize chain so the PE clock stays at 2.4GHz
        # into the output-projection epilogue
        for _ in range(20):
            pp = psP.tile([128, 512], F32, tag="pp", name="ppd")
            nc.tensor.matmul(pp[0:DV + 1, :], vt[:, 0, 0, :],
                             qt[:, 0, 0:512], start=True, stop=True)
        # O(2) has no dependence on the last normalize, so interleaving it
        # with O(3) fills the PE while the final softmax normalize drains
        for i in range(4):
            for n in range(2):
                o_unit(8 + i, n)()
            for n in range(2):
                o_unit(12 + i, n)()

    nc.compile()
    return nc


def make_in_maps(x, Wq, Wk, Wv, Wo):
    bf = ml_dtypes.bfloat16
    in_maps = []
    for c in range(N_CORES):
        b, g = c // 2, c % 2
        xtb = np.ascontiguousarray(
            x[b].T.reshape(8, 128, L).transpose(1, 0, 2)).astype(bf)
        wqg = np.ascontiguousarray(
            Wq[:, g * OC:(g + 1) * OC].reshape(8, 128, OC)
            .transpose(1, 0, 2)).astype(bf)
        wkg = np.ascontiguousarray(
            Wk[:, g * OC:(g + 1) * OC].reshape(8, 128, OC)
            .transpose(1, 0, 2)).astype(bf)
        wvg = np.ascontiguousarray(
            Wv[:, g * OC:(g + 1) * OC].reshape(8, 128, OC)
            .transpose(1, 0, 2)).astype(bf)
        wog = np.ascontiguousarray(
            Wo[g * OC:(g + 1) * OC, :].reshape(4, 128, D)
            .transpose(1, 0, 2)).astype(bf)
        in_maps.append({"xt": xtb, "wq": wqg, "wk": wkg, "wv": wvg,
                        "wo": wog})
    return in_maps


_NC_CACHE = {}


def _get_nc():
    if "nc" not in _NC_CACHE:
        _NC_CACHE["nc"] = build_nc()
    return _NC_CACHE["nc"]


def _numpy_fallback(x, Wq, Wk, Wv, Wo, bo, mask):
    Bsz, Lq, _ = x.shape
    Q = (x @ Wq).reshape(Bsz, Lq, N_HEAD, DH).transpose(0, 2, 1, 3)
    K = (x @ Wk).reshape(Bsz, Lq, N_HEAD, DH).transpose(0, 2, 1, 3)
    V = (x @ Wv).reshape(Bsz, Lq, N_HEAD, DV).transpose(0, 2, 1, 3)
    s = np.einsum("bhqd,bhkd->bhqk", Q, K) / np.sqrt(np.float32(DH))
    s = np.where(mask, s, -np.inf)
    s = s - s.max(axis=-1, keepdims=True)
    p = np.exp(s)
    p /= p.sum(axis=-1, keepdims=True)
    ctxv = np.einsum("bhqk,bhkv->bhqv", p, V)
    ctxv = ctxv.transpose(0, 2, 1, 3).reshape(Bsz, Lq, N_HEAD * DV)
    return (ctxv @ Wo + bo).astype(np.float32)


def run_on_hw(in_maps, trace=False):
    from concourse.bass_utils import run_bass_kernel_spmd
    nc = _get_nc()
    return run_bass_kernel_spmd(nc, in_maps, list(range(N_CORES)),
                                trace=trace)


def kernel(x, Wq, Wk, Wv, Wo, bo, mask, _trace=False, _results=None):
    x = np.asarray(x, dtype=np.float32)
    Wq = np.asarray(Wq, dtype=np.float32)
    Wk = np.asarray(Wk, dtype=np.float32)
    Wv = np.asarray(Wv, dtype=np.float32)
    Wo = np.asarray(Wo, dtype=np.float32)
    bo = np.asarray(bo, dtype=np.float32)
    mask_np = np.asarray(mask).reshape(mask.shape[-2], mask.shape[-1])

    causal = bool(np.array_equal(
        mask_np, np.tril(np.ones((L, L), dtype=bool))))
    if not causal or x.shape != (B, L, D):
        return _numpy_fallback(np.asarray(x), Wq, Wk, Wv, Wo, bo,
                               np.asarray(mask))

    res = run_on_hw(make_in_maps(x, Wq, Wk, Wv, Wo), trace=_trace)
    if _results is not None:
        _results.append(res)
    out = np.empty((B, L, D), dtype=np.float32)
    for b in range(B):
        out[b] = res.results[2 * b]["out"] + res.results[2 * b + 1]["out"] + bo
    return out
